# revision 30
# baseline (speedup 1.0000x reference)
"""Trainium2 Bass kernel for prefix-KV multi-head attention (v2).

Reference computation (per batch):
    qkv = x @ w_qkv -> q,k,v heads; k/v get a 16-token prefix (pk, pv)
    attn = softmax(q @ k^T * D^-0.5); out = (attn @ v) @ w_proj + b_proj

Sharding: data-parallel over B across 8 NeuronCores (2 batches per core).

v2 design (vs v1):
  - weights loaded to SBUF once per core (bf16), reused by both batches
  - q^T kept in SBUF (no DRAM spill)
  - v computed in NATURAL [token, feature] layout via x^T-stationary GEMM
    (moving = w_v columns), eliminating all per-head v transposes
  - x^T built with bf16 PE transposes (2x faster than fp32)
  - attention runs per HEAD (not head-pair): PSUM = scores 2x2 banks
    (double buffered) + av accumulator 2 banks + gemm scratch 2x1 banks
    = 8 banks exactly
  - q/k/v GEMM chunks for pair p+1 and proj passes of the previous batch
    are software-pipelined into the attention mt-loop slots, so the PE
    stays busy while ACT computes exp()
  - softmax normalization uses reciprocal_approx_fast (51 ULP) instead
    of the 6.5us iterative DVE reciprocal
  - ones-columns packed next to v in v_ext give the softmax denominator
    for free inside the attention@v matmul (rows 64:128 of the psum)

This file is self-contained: it monkeypatches two workarounds for the
walrus build in this container (1-sync-wait-per-instruction cap).
"""

import json
import os
import sys
from collections import deque

for _p in ("/opt/trn_rl_repo", os.path.expanduser("~/.axon_site/_ro/trn_rl_repo")):
    if os.path.isdir(_p) and _p not in sys.path:
        sys.path.insert(0, _p)

import numpy as np

import concourse.bass as bass
import concourse.tile as tile
from concourse import mybir
from concourse.bass_utils import run_bass_kernel_spmd
from concourse.vector_clock import ScopedClock
from concourse.masks import make_identity

F32 = mybir.dt.float32
BF16 = mybir.dt.bfloat16
AF = mybir.ActivationFunctionType

# ---------------------------------------------------------------------------
# Workaround: this container's walrus supports at most ONE sync wait per
# instruction.  (a) split the TileContext-exit drain's waits onto single-wait
# NOPs; (b) at BIR-JSON serialization time, hoist extra waits from any
# instruction onto same-engine NOPs placed immediately before it.
# ---------------------------------------------------------------------------

def _patched_drain_and_barrier(self, tick_clock, wait_clock):
    drain_inst = self.nc.sync.drain()
    wait_clock.add_sem_waits(
        drain_inst.ins, ScopedClock({None: tick_clock.global_clock})
    )
    si = drain_inst.ins.sync_info
    waits = list(si.on_wait) if si is not None and si.on_wait else []
    if len(waits) > 1:
        si.on_wait = waits[:1]
        for w in waits[1:]:
            nop = self.nc.sync.nop(hint="drain_wait_split", nofuse=True)
            nsi = nop.ins.sync_info
            if nsi is None:
                nop.ins.sync_info = mybir.SyncInfo(on_wait=[w], on_update=[])
            else:
                nsi.on_wait = list(nsi.on_wait or []) + [w]
    self.nc.all_engine_barrier()
    assert self.sems is not None
    popped = self.nc._tile_sem_poison_stack.pop()
    assert popped is self._sem_poison
    self.nc.clear_and_free_semaphores(list(self.sems.allocated().values()))
    self.nc.all_engine_barrier()


tile.TileContext._drain_and_barrier = _patched_drain_and_barrier


def _split_multi_waits(bir):
    for fn in bir["functions"]:
        for bb in fn["blocks"]:
            new_insts = []
            for inst in bb["instructions"]:
                si = inst.get("sync_info")
                ow = (si or {}).get("on_wait") or []
                if len(ow) > 1:
                    for i, w in enumerate(ow[:-1]):
                        new_insts.append({
                            "debug": inst.get("debug", 0),
                            "engine": inst["engine"],
                            "ins": [], "outs": [],
                            "name": f"{inst['name']}.wsplit{i}",
                            "opcode": "NoOp",
                            "sync_info": {"on_wait": [w], "on_update": []},
                        })
                    si["on_wait"] = [ow[-1]]
                new_insts.append(inst)
            bb["instructions"] = new_insts
    return bir


_orig_to_json_bytes = bass.Bass.to_json_bytes


def _patched_to_json_bytes(self):
    d = json.loads(_orig_to_json_bytes(self))
    _split_multi_waits(d)
    return json.dumps(d).encode()


bass.Bass.to_json_bytes = _patched_to_json_bytes

# ---------------------------------------------------------------------------
# Problem constants (hardcoded per the task contract)
# ---------------------------------------------------------------------------

B, N, C, H, P = 16, 1024, 1024, 16, 16
D = C // H                      # 64
SCALE = float(D) ** -0.5        # 0.125
N_CORES = 8
B_PC = B // N_CORES             # 2 batches per core
NT = N // 128                   # 8 token tiles
CT = C // 128                   # 8 feature tiles
MT = NT + 1                     # 9 m-tiles: tile 0 = prefix (16 valid rows)
HPAIRS = H // 2                 # 8 head pairs


def build_nc(repeat: int = 1) -> bass.Bass:
    nc = bass.Bass()

    x_d = nc.declare_dram_parameter("x", [B_PC, N, C], F32, isOutput=False)
    pk_d = nc.declare_dram_parameter("pk", [B_PC, P, C], F32, isOutput=False)
    pv_d = nc.declare_dram_parameter("pv", [B_PC, P, C], F32, isOutput=False)
    wqkv_d = nc.declare_dram_parameter("w_qkv", [C, 3 * C], F32, isOutput=False)
    wproj_d = nc.declare_dram_parameter("w_proj", [C, C], F32, isOutput=False)
    bias_d = nc.declare_dram_parameter("b_proj", [C], F32, isOutput=False)
    # output is stored TRANSPOSED per batch: [C, N]; host transposes back
    outT_d = nc.declare_dram_parameter("outT", [B_PC, C, N], F32, isOutput=True)

    with tile.TileContext(nc) as tc:
        with tc.tile_pool(name="cons", bufs=1) as cons, \
             tc.tile_pool(name="xload", bufs=2) as xload, \
             tc.tile_pool(name="xbf", bufs=1) as xbfp, \
             tc.tile_pool(name="eP", bufs=3) as e_pool, \
             tc.tile_pool(name="stg", bufs=1) as stg, \
             tc.tile_pool(name="rbp", bufs=1) as rb_pool, \
             tc.tile_pool(name="swp", bufs=2) as swp, \
             tc.tile_pool(name="osb", bufs=2) as osb, \
             tc.tile_pool(name="psS", bufs=2, space="PSUM") as psS, \
             tc.tile_pool(name="psAV", bufs=1, space="PSUM") as psAV, \
             tc.tile_pool(name="psG", bufs=2, space="PSUM") as psG:

            # ---------------- one-time setup ----------------
            ident_bf = cons.tile([128, 128], BF16, tag="idb")
            make_identity(nc, ident_bf[:])
            ident_f = cons.tile([128, 128], F32, tag="idf")
            make_identity(nc, ident_f[:])
            # bias in per-partition layout: bias_col[p, cf] = b_proj[cf*128+p]
            bias_col = cons.tile([128, CT], F32, tag="bias")
            nc.sync.dma_start(
                out=bias_col[:],
                in_=bias_d[:].rearrange("(a b) -> b a", b=128),
            )
            # weights, bf16, resident for the whole kernel. Loaded in
            # priority order (single sw-DGE queue serializes in emission
            # order): pair-0 q/k first, then v block 0, then the rest, so
            # batch 0's first GEMMs start ~3us in instead of ~50us.
            wq_sb = cons.tile([128, CT, C], BF16, tag="wq")
            wk_sb = cons.tile([128, CT, C], BF16, tag="wk")
            wv_sb = cons.tile([128, CT, C], BF16, tag="wv")
            wp_sb = cons.tile([128, CT, C], BF16, tag="wp")

            def _wload(dst, base, lo, hi):
                nc.gpsimd.dma_start(
                    out=dst[:, :, lo:hi],
                    in_=wqkv_d[:, base + lo:base + hi].rearrange(
                        "(ct p) f -> p ct f", p=128),
                )

            _wload(wk_sb, C, 0, 128)                  # k pair 0
            _wload(wq_sb, 0, 0, 128)                  # q pair 0
            _wload(wv_sb, 2 * C, 0, 512)              # v block 0
            _wload(wv_sb, 2 * C, 512, 1024)           # v block 1
            for p in range(1, HPAIRS):
                _wload(wk_sb, C, p * 128, (p + 1) * 128)
                _wload(wq_sb, 0, p * 128, (p + 1) * 128)
            nc.gpsimd.dma_start(
                out=wp_sb[:],
                in_=wproj_d[:].rearrange("(ct p) f -> p ct f", p=128),
            )

            # persistent activations (reused across batches; Tile tracks
            # read/write hazards on AP ranges)
            xT = cons.tile([128, CT, N], BF16, tag="xT")
            kT = cons.tile([128, CT, MT * 128], BF16, tag="kT")
            qT = cons.tile([128, CT, N], BF16, tag="qT")
            oT = cons.tile([128, CT, N], BF16, tag="oT")
            # v_ext[m, mt, h, 0:64] = v values; [.., 64:128] = ones columns
            # (denominator trick). m-tile 0 = prefix: rows 0:16 valid, rows
            # 16:128 ALL-ZERO so they contribute nothing to out or denom.
            v_ext = cons.tile([128, MT, H, 128], BF16, tag="vx")
            nc.vector.memset(v_ext[:, :, :, 64:128], 1.0)
            # prefix m-tile: zero everything (APs must start at partition
            # 0/32/64/96, so zero all 128 rows then re-set ones on rows 0:16)
            nc.vector.memset(v_ext[:, 0, :, :], 0.0)
            nc.vector.memset(v_ext[0:P, 0, :, 64:128], 1.0)
            # prefix pad columns of k: zero keys -> e = exp(0) = 1, harmless
            # because the matching v_ext rows are zero
            nc.vector.memset(kT[:, :, P:128], 0.0)

            # ---------------- per-batch work units ----------------

            # PE tile-packing hazard guard: a full-array (K=128) matmul
            # issued while two CONCURRENT row-half tiled matmuls are still
            # streaming corrupts the array (observed on HW; CoreSim-clean).
            # Any full-array matmul that could issue right behind a packed
            # pair takes an explicit sync dep on the last packed matmul.
            last_packed = [None]

            def _pack_guard(mm):
                if last_packed[0] is not None:
                    bass._add_dep_helper(
                        mm.ins, last_packed[0].ins, sync=True,
                        reason="drain packed tiles before full-array matmul",
                    )
                    last_packed[0] = None
                return mm

            swaps_by_pair = {}

            def qk_units(b, p):
                """4 closures: q and k GEMMs for head pair p, split in two
                512-column halves each. Each accumulates 8 c-tiles into a
                [128,512] psum and copies (cast bf16) into qT/kT, PLUS
                partition-swapped copies (rows 0:64 <-> 64:128): PE tile
                packing overlaps two K=64 matmuls only when consecutive
                instructions sit in different row-halves of the array, so
                each head's odd m-tiles read k/q at the opposite offset.
                ksw holds odd m-tiles 1,3,5,7 only; qsw is fully swapped."""
                ksw = swp.tile([128, 512], BF16, tag="ksw",
                               name=f"ksw_{b}_{p}")
                qsw = swp.tile([128, N], BF16, tag="qsw",
                               name=f"qsw_{b}_{p}")
                swaps_by_pair[p] = (ksw, qsw)
                us = []
                for which in ("k", "q"):
                    for jh in range(2):
                        def u(which=which, p=p, jh=jh, b=b):
                            w_sb = wk_sb if which == "k" else wq_sb
                            ps = psG.tile([128, 512], F32, tag="g",
                                          name=f"g{which}_{b}_{p}_{jh}")
                            for ct in range(CT):
                                mm = nc.tensor.matmul(
                                    ps[:],
                                    w_sb[:, ct, p * 128:(p + 1) * 128],
                                    xT[:, ct, jh * 512:(jh + 1) * 512],
                                    start=(ct == 0), stop=(ct == CT - 1),
                                )
                                if ct == 0:
                                    _pack_guard(mm)
                            if which == "k":
                                nc.vector.tensor_copy(
                                    kT[:, p, 128 + jh * 512:128 + (jh + 1) * 512],
                                    ps[:],
                                )
                                # psum col block i2*128 is token m-tile
                                # mt = 1 + jh*4 + i2; keep the odd ones
                                for i2 in (0, 2):
                                    i = (jh * 4 + i2) // 2
                                    for half in (0, 1):
                                        nc.vector.tensor_copy(
                                            ksw[(1 - half) * 64:
                                                (2 - half) * 64,
                                                i * 128:(i + 1) * 128],
                                            ps[half * 64:(half + 1) * 64,
                                               i2 * 128:(i2 + 1) * 128],
                                        )
                            else:
                                nc.vector.tensor_copy(
                                    qT[:, p, jh * 512:(jh + 1) * 512], ps[:]
                                )
                                for half in (0, 1):
                                    nc.vector.tensor_copy(
                                        qsw[(1 - half) * 64:(2 - half) * 64,
                                            jh * 512:(jh + 1) * 512],
                                        ps[half * 64:(half + 1) * 64, :],
                                    )
                        us.append(u)
                return us

            def v_units(b, bk):
                """8 closures: v GEMM for pair block bk (4 pairs = 512 v
                columns), one per token tile. x^T tile is stationary, w_v
                columns are moving -> v lands in NATURAL [token, feature]
                layout, no transpose needed."""
                us = []
                for nt in range(NT):
                    def u(nt=nt, bk=bk, b=b):
                        ps = psG.tile([128, 512], F32, tag="g",
                                      name=f"gv_{b}_{bk}_{nt}")
                        for ct in range(CT):
                            mm = nc.tensor.matmul(
                                ps[:],
                                xT[:, ct, nt * 128:(nt + 1) * 128],
                                wv_sb[:, ct, bk * 512:(bk + 1) * 512],
                                start=(ct == 0), stop=(ct == CT - 1),
                            )
                            if ct == 0:
                                _pack_guard(mm)
                        nc.vector.tensor_copy(
                            v_ext[:, nt + 1, 8 * bk:8 * (bk + 1), 0:64],
                            ps[:].rearrange("p (h d) -> p h d", d=64),
                        )
                    us.append(u)
                return us

            def proj_units(b):
                """8 closures: one projection f-tile pass each; emitted
                interleaved into the NEXT batch's preamble."""
                us = []
                for cf in range(CT):
                    def u(cf=cf, b=b):
                        ps = psS.tile([128, N], F32, tag="s",
                                      name=f"pp_{b}_{cf}")
                        for ct in range(CT):
                            for j in (0, 512):
                                mm = nc.tensor.matmul(
                                    ps[:, j:j + 512],
                                    wp_sb[:, ct, cf * 128:(cf + 1) * 128],
                                    oT[:, ct, j:j + 512],
                                    start=(ct == 0), stop=(ct == CT - 1),
                                )
                                if ct == 0 and j == 0:
                                    _pack_guard(mm)
                        o_sb = osb.tile([128, N], F32, tag="o",
                                        name=f"osb_{b}_{cf}")
                        nc.vector.tensor_scalar_add(
                            o_sb[:], ps[:], bias_col[:, cf:cf + 1]
                        )
                        nc.sync.dma_start(
                            out=outT_d[b, cf * 128:(cf + 1) * 128, :],
                            in_=o_sb[:],
                        )
                    us.append(u)
                return us

            def emit_batch(b, carry):
                """Emit one batch; `carry` = proj closures of the previous
                batch, interleaved into this batch's preamble. Returns this
                batch's proj closures."""
                units = deque(carry)

                def drain(k=1):
                    for _ in range(k):
                        if units:
                            units.popleft()()

                # ---- preamble: x^T via bf16 PE transposes; the v GEMM for
                # token tile nt only needs xT columns nt*128..(nt+1)*128, so
                # it runs right behind transpose nt and fills the PE while
                # the next DMA/cast completes ----
                vb0 = v_units(b, 0)
                for nt in range(NT):
                    xl = xload.tile([128, C], F32, tag="xl",
                                    name=f"xl_{b}_{nt}")
                    nc.sync.dma_start(
                        out=xl[:], in_=x_d[b, nt * 128:(nt + 1) * 128, :]
                    )
                    xbf = xbfp.tile([128, C], BF16, tag="xbf",
                                    name=f"xbf_{b}_{nt}")
                    nc.scalar.activation(xbf[:], xl[:], AF.Copy)
                    ps_t = psG.tile([128, CT, 128], BF16, tag="g",
                                    name=f"pst_{b}_{nt}")
                    for ct in range(CT):
                        tr = nc.tensor.transpose(
                            ps_t[:, ct, :],
                            xbf[:, ct * 128:(ct + 1) * 128],
                            ident_bf[:],
                        )
                        if ct == 0:
                            _pack_guard(tr)
                    nc.vector.tensor_copy(
                        xT[:, :, nt * 128:(nt + 1) * 128], ps_t[:]
                    )
                    vb0[nt]()
                    drain(1)

                # ---- prefix: pk^T into kT cols 0:16; pv into v_ext ----
                pkl = xload.tile([128, C], F32, tag="xl", name=f"pkl_{b}")
                nc.sync.dma_start(out=pkl[0:P, :], in_=pk_d[b])
                ps_pk = psG.tile([128, CT, P], F32, tag="g",
                                 name=f"pspk_{b}")
                for ct in range(CT):
                    nc.tensor.transpose(
                        ps_pk[:, ct, :],
                        pkl[0:P, ct * 128:(ct + 1) * 128],
                        ident_f[0:P, 0:P],
                    )
                nc.vector.tensor_copy(kT[:, :, 0:P], ps_pk[:])
                nc.gpsimd.dma_start(
                    out=v_ext[0:P, 0, :, 0:64],
                    in_=pv_d[b].rearrange("p (h d) -> p h d", d=64),
                )
                drain(1)

                # ---- pair-0 q/k, interleaving the carry ----
                for u in qk_units(b, 0):
                    u()
                    drain(1)
                drain(len(units))  # force out any remaining carry

                # ---- per-head attention, gemm pipeline in the slots ----
                queue = deque()
                for p in range(HPAIRS):
                    if p + 1 < HPAIRS:
                        queue.extend(qk_units(b, p + 1))
                    if p == 0:
                        queue.extend(v_units(b, 1))
                    ksw, qsw = swaps_by_pair[p]
                    slot = 0
                    for hh in range(2):
                        base = hh * 64          # even m-tiles: native offset
                        base_o = 64 - base      # odd m-tiles: swapped offset
                        h = 2 * p + hh
                        ps_av = psAV.tile([128, N], F32, tag="av",
                                          name=f"av_{b}_{h}")
                        for mg in range(0, MT, 2):
                            grp = (mg,) if mg == MT - 1 else (mg, mg + 1)
                            tiles = []
                            for mt in grp:
                                tiles.append(e_pool.tile(
                                    [128, N], BF16, tag="e",
                                    name=f"e_{b}_{h}_{mt}"))
                            pss = [psS.tile([128, N], F32, tag="s",
                                            name=f"s_{b}_{h}_{mt}")
                                   for mt in grp]
                            # packed scores: consecutive matmuls alternate
                            # row-halves (even mt at `base` from kT/qT, odd
                            # mt at `base_o` from the swapped views), so the
                            # PE overlaps them pairwise (~1.9x)
                            for j in (0, 512):
                                for sl, mt in enumerate(grp):
                                    if mt % 2 == 0:
                                        nc.tensor.matmul(
                                            pss[sl][:, j:j + 512],
                                            kT[base:base + D, p,
                                               mt * 128:(mt + 1) * 128],
                                            qT[base:base + D, p, j:j + 512],
                                            start=True, stop=True,
                                        )
                                    else:
                                        i = (mt - 1) // 2
                                        last_packed[0] = nc.tensor.matmul(
                                            pss[sl][:, j:j + 512],
                                            ksw[base_o:base_o + D,
                                                i * 128:(i + 1) * 128],
                                            qsw[base_o:base_o + D,
                                                j:j + 512],
                                            start=True, stop=True,
                                        )
                            for sl, mt in enumerate(grp):
                                nc.scalar.activation(tiles[sl][:],
                                                     pss[sl][:], AF.Exp,
                                                     scale=SCALE)
                            # gemm/proj filler BETWEEN exps and avs: covers
                            # the exp latency and, at mg==0, the previous
                            # head's psum release
                            slot += 1
                            if queue and (mg in (0, 4)
                                          or len(queue) >= 10 - slot):
                                queue.popleft()()
                            for sl, mt in enumerate(grp):
                                for j in (0, 512):
                                    nc.tensor.matmul(
                                        ps_av[:, j:j + 512],
                                        v_ext[:, mt, h, :],
                                        tiles[sl][:, j:j + 512],
                                        start=(mt == 0),
                                        stop=(mt == MT - 1),
                                    )
                        # normalize: out = unnorm * exp(-ln(denom)).
                        # (custom-DVE reciprocal_approx is unsupported by this
                        # walrus; iterative DVE reciprocal costs 6.5us.)
                        # The numerator is copied to SBUF so the psum
                        # accumulator is released after ~1.1us (copy || ln)
                        # instead of after the full ln->exp->mul chain.
                        num_sb = stg.tile([64, N], BF16, tag="st",
                                          name=f"st_{b}_{h}")
                        nc.vector.tensor_copy(num_sb[:], ps_av[0:64, :])
                        lnd = rb_pool.tile([64, N], F32, tag="ln",
                                           name=f"ln_{b}_{h}")
                        nc.scalar.activation(lnd[:], ps_av[64:128, :], AF.Ln)
                        rb = rb_pool.tile([64, N], BF16, tag="rb",
                                          name=f"rb_{b}_{h}")
                        nc.scalar.activation(rb[:], lnd[:], AF.Exp,
                                             scale=-1.0)
                        nc.vector.tensor_mul(
                            oT[base:base + D, p, :], num_sb[:], rb[:]
                        )
                    while queue:
                        queue.popleft()()

                return proj_units(b)

            carry = []
            for _rep in range(repeat):
                for b in range(B_PC):
                    carry = emit_batch(b, carry)
            for u in carry:
                u()

    return nc


_NC_CACHE = {}


def _get_nc(repeat: int = 1) -> bass.Bass:
    key = f"nc{repeat}"
    if key not in _NC_CACHE:
        _NC_CACHE[key] = build_nc(repeat)
    return _NC_CACHE[key]


def _make_runner(nc):
    """Compile the SPMD kernel ONCE into a reusable callable.

    Mirrors bass2jax.run_bass_via_pjrt's multi-core branch, but without
    output-buffer donation so the compiled function + device-resident
    inputs can be invoked repeatedly (for wall-clock benchmarking and to
    avoid recompiles on every kernel() call).
    """
    import jax
    from jax.experimental.shard_map import shard_map
    from jax.sharding import Mesh, PartitionSpec
    from concourse import bass2jax
    from concourse.bass2jax import _bass_exec_p, partition_id_tensor

    bass2jax.install_neuronx_cc_hook()

    partition_name = (
        nc.partition_id_tensor.name if nc.partition_id_tensor else None
    )
    in_names, out_names, out_avals, zero_outs = [], [], [], []
    for alloc in nc.m.functions[0].allocations:
        if not isinstance(alloc, mybir.MemoryLocationSet):
            continue
        name = alloc.memorylocations[0].name
        if alloc.kind == "ExternalInput":
            if name != partition_name:
                in_names.append(name)
        elif alloc.kind == "ExternalOutput":
            shape = tuple(alloc.tensor_shape)
            dtype = mybir.dt.np(alloc.dtype)
            out_names.append(name)
            out_avals.append(jax.core.ShapedArray(shape, dtype))
            zero_outs.append(np.zeros(shape, dtype))
    n_params = len(in_names)
    all_in_names = list(in_names) + list(out_names)
    if partition_name is not None:
        all_in_names.append(partition_name)

    def _body(*args):
        operands = list(args)
        if partition_name is not None:
            operands.append(partition_id_tensor())
        outs = _bass_exec_p.bind(
            *operands,
            out_avals=tuple(out_avals),
            in_names=tuple(all_in_names),
            out_names=tuple(out_names),
            lowering_input_output_aliases=(),
            sim_require_finite=True,
            sim_require_nnan=True,
            nc=nc,
        )
        return tuple(outs)

    devices = jax.devices()[:N_CORES]
    mesh = Mesh(np.asarray(devices), ("core",))
    n_outs = len(out_avals)
    in_specs = (PartitionSpec("core"),) * (n_params + n_outs)
    out_specs = (PartitionSpec("core"),) * n_outs
    sharded = jax.jit(
        shard_map(_body, mesh=mesh, in_specs=in_specs,
                  out_specs=out_specs, check_rep=False),
        keep_unused=True,
    )

    concat_zeros = [
        np.zeros((N_CORES * z.shape[0], *z.shape[1:]), z.dtype)
        for z in zero_outs
    ]

    state = {"dev_zeros": None}

    def runner(in_maps):
        per_core = [
            [np.asarray(m[name]) for name in in_names] for m in in_maps
        ]
        concat_in = [
            np.concatenate([per_core[c][i] for c in range(N_CORES)], axis=0)
            for i in range(n_params)
        ]
        if state["dev_zeros"] is None:
            state["dev_zeros"] = [jax.device_put(z) for z in concat_zeros]
        out_arrs = sharded(*concat_in, *state["dev_zeros"])
        return [
            {
                name: np.asarray(out_arrs[i]).reshape(
                    N_CORES, *out_avals[i].shape
                )[c]
                for i, name in enumerate(out_names)
            }
            for c in range(N_CORES)
        ]

    def runner_dev(dev_args):
        """dev_args: device-resident concat inputs; returns device outputs."""
        return sharded(*dev_args, *state["dev_zeros"])

    def make_dev_args(in_maps):
        per_core = [
            [np.asarray(m[name]) for name in in_names] for m in in_maps
        ]
        concat_in = [
            np.concatenate([per_core[c][i] for c in range(N_CORES)], axis=0)
            for i in range(n_params)
        ]
        if state["dev_zeros"] is None:
            state["dev_zeros"] = [jax.device_put(z) for z in concat_zeros]
        return [jax.device_put(a) for a in concat_in]

    return runner, runner_dev, make_dev_args


def _get_runner(repeat: int = 1):
    key = f"runner{repeat}"
    if key not in _NC_CACHE:
        _NC_CACHE[key] = _make_runner(_get_nc(repeat))
    return _NC_CACHE[key]


def _make_in_maps(x, pk, pv, w_qkv, w_proj, b_proj):
    x = np.ascontiguousarray(np.asarray(x, dtype=np.float32))
    pk = np.ascontiguousarray(np.asarray(pk, dtype=np.float32))
    pv = np.ascontiguousarray(np.asarray(pv, dtype=np.float32))
    w_qkv = np.ascontiguousarray(np.asarray(w_qkv, dtype=np.float32))
    w_proj = np.ascontiguousarray(np.asarray(w_proj, dtype=np.float32))
    b_proj = np.ascontiguousarray(np.asarray(b_proj, dtype=np.float32))
    in_maps = []
    for c in range(N_CORES):
        sl = slice(c * B_PC, (c + 1) * B_PC)
        in_maps.append({
            "x": x[sl], "pk": pk[sl], "pv": pv[sl],
            "w_qkv": w_qkv, "w_proj": w_proj, "b_proj": b_proj,
        })
    return in_maps


def run(x, pk, pv, w_qkv, w_proj, b_proj, trace=False, **trace_kwargs):
    """Run the SPMD kernel; returns (output [B,N,C], results).

    With trace=True, routes through run_bass_kernel_spmd so the returned
    results object carries .exec_time_ns / .profile_json.
    """
    in_maps = _make_in_maps(x, pk, pv, w_qkv, w_proj, b_proj)
    if trace:
        res = run_bass_kernel_spmd(
            _get_nc(), in_maps, list(range(N_CORES)), trace=True,
            **trace_kwargs,
        )
        results = res.results
        out = np.empty((B, N, C), dtype=np.float32)
        for c in range(N_CORES):
            outT = results[c]["outT"]          # [B_PC, C, N]
            out[c * B_PC:(c + 1) * B_PC] = outT.transpose(0, 2, 1)
        return out, res
    runner, _, _ = _get_runner()
    results = runner(in_maps)
    out = np.empty((B, N, C), dtype=np.float32)
    for c in range(N_CORES):
        outT = results[c]["outT"]              # [B_PC, C, N]
        out[c * B_PC:(c + 1) * B_PC] = outT.transpose(0, 2, 1)
    return out, results


def kernel(x, pk, pv, w_qkv, w_proj, b_proj) -> np.ndarray:
    out, _ = run(x, pk, pv, w_qkv, w_proj, b_proj)
    return out


def benchmark(x, pk, pv, w_qkv, w_proj, b_proj, iters=20, warmup=3, repeat=1):
    """Median wall-clock per executed call with device-resident inputs."""
    import time
    import jax
    _, runner_dev, make_dev_args = _get_runner(repeat)
    in_maps = _make_in_maps(x, pk, pv, w_qkv, w_proj, b_proj)
    dev_args = make_dev_args(in_maps)
    for _ in range(warmup):
        outs = runner_dev(dev_args)
        jax.block_until_ready(outs)
    ts = []
    for _ in range(iters):
        t0 = time.perf_counter()
        outs = runner_dev(dev_args)
        jax.block_until_ready(outs)
        ts.append(time.perf_counter() - t0)
    ts.sort()
    return {
        "median_s": ts[len(ts) // 2],
        "min_s": ts[0],
        "all_s": ts,
    }


# revision 32
# speedup vs baseline: 1.0354x; 1.0354x over previous
"""Trainium2 Bass kernel for prefix-KV multi-head attention (v2).

Reference computation (per batch):
    qkv = x @ w_qkv -> q,k,v heads; k/v get a 16-token prefix (pk, pv)
    attn = softmax(q @ k^T * D^-0.5); out = (attn @ v) @ w_proj + b_proj

Sharding: data-parallel over B across 8 NeuronCores (2 batches per core).

Design (vs the v1 baseline, 700us -> 608us):
  - weights loaded to SBUF once per core (bf16), priority-ordered DMA
    (pair-0 q/k first), reused by both batches
  - q^T kept in SBUF (no DRAM spill)
  - v computed in NATURAL [token, feature] layout via x^T-stationary GEMM
    (moving = w_v columns), eliminating all per-head v transposes
  - x^T built with bf16 PE transposes (2x faster than fp32)
  - attention runs per HEAD (not head-pair): PSUM = scores 2x2 banks
    (double buffered) + av accumulator 2 banks + gemm scratch 2x1 banks
    = 8 banks exactly
  - q/k/v GEMM chunks for pair p+1 and proj passes of the previous batch
    are software-pipelined into the attention mt-loop slots (emitted
    between exp and av so they also cover the exp latency and the
    previous head's psum release), keeping the PE busy while ACT exps
  - softmax 1/denominator via exp(-ln(d)) on ACT (this walrus lacks the
    custom-DVE approx ops; iterative DVE reciprocal costs 6.5us); a DVE
    copy of the numerator releases the av psum accumulator early
  - ones-columns packed next to v in v_ext give the softmax denominator
    for free inside the attention@v matmul (rows 64:128 of the psum)

Explored and rejected: fp8 (tolerance 2e-2 exceeded: random-sign GEMM
error stays ~5.7% relative), PE tile-packing of the K=64 score matmuls
(verified ~1.9x on adjacent alternating-row-half matmuls, but a
full-array matmul issued behind a packed pair corrupts the array unless
sync-guarded, and with guards + the exp-paced pipeline it measured
slower: 641us packed vs 608us unpacked; see kernel_v5.py).

This file is self-contained: it monkeypatches two workarounds for the
walrus build in this container (1-sync-wait-per-instruction cap).
"""

import json
import os
import sys
from collections import deque

for _p in ("/opt/trn_rl_repo", os.path.expanduser("~/.axon_site/_ro/trn_rl_repo")):
    if os.path.isdir(_p) and _p not in sys.path:
        sys.path.insert(0, _p)

import numpy as np

import concourse.bass as bass
import concourse.tile as tile
from concourse import mybir
from concourse.bass_utils import run_bass_kernel_spmd
from concourse.vector_clock import ScopedClock
from concourse.masks import make_identity

F32 = mybir.dt.float32
BF16 = mybir.dt.bfloat16
AF = mybir.ActivationFunctionType

# ---------------------------------------------------------------------------
# Workaround: this container's walrus supports at most ONE sync wait per
# instruction.  (a) split the TileContext-exit drain's waits onto single-wait
# NOPs; (b) at BIR-JSON serialization time, hoist extra waits from any
# instruction onto same-engine NOPs placed immediately before it.
# ---------------------------------------------------------------------------

def _patched_drain_and_barrier(self, tick_clock, wait_clock):
    drain_inst = self.nc.sync.drain()
    wait_clock.add_sem_waits(
        drain_inst.ins, ScopedClock({None: tick_clock.global_clock})
    )
    si = drain_inst.ins.sync_info
    waits = list(si.on_wait) if si is not None and si.on_wait else []
    if len(waits) > 1:
        si.on_wait = waits[:1]
        for w in waits[1:]:
            nop = self.nc.sync.nop(hint="drain_wait_split", nofuse=True)
            nsi = nop.ins.sync_info
            if nsi is None:
                nop.ins.sync_info = mybir.SyncInfo(on_wait=[w], on_update=[])
            else:
                nsi.on_wait = list(nsi.on_wait or []) + [w]
    self.nc.all_engine_barrier()
    assert self.sems is not None
    popped = self.nc._tile_sem_poison_stack.pop()
    assert popped is self._sem_poison
    self.nc.clear_and_free_semaphores(list(self.sems.allocated().values()))
    self.nc.all_engine_barrier()


tile.TileContext._drain_and_barrier = _patched_drain_and_barrier


def _split_multi_waits(bir):
    for fn in bir["functions"]:
        for bb in fn["blocks"]:
            new_insts = []
            for inst in bb["instructions"]:
                si = inst.get("sync_info")
                ow = (si or {}).get("on_wait") or []
                if len(ow) > 1:
                    for i, w in enumerate(ow[:-1]):
                        new_insts.append({
                            "debug": inst.get("debug", 0),
                            "engine": inst["engine"],
                            "ins": [], "outs": [],
                            "name": f"{inst['name']}.wsplit{i}",
                            "opcode": "NoOp",
                            "sync_info": {"on_wait": [w], "on_update": []},
                        })
                    si["on_wait"] = [ow[-1]]
                new_insts.append(inst)
            bb["instructions"] = new_insts
    return bir


_orig_to_json_bytes = bass.Bass.to_json_bytes


def _patched_to_json_bytes(self):
    d = json.loads(_orig_to_json_bytes(self))
    _split_multi_waits(d)
    return json.dumps(d).encode()


bass.Bass.to_json_bytes = _patched_to_json_bytes

# ---------------------------------------------------------------------------
# Problem constants (hardcoded per the task contract)
# ---------------------------------------------------------------------------

B, N, C, H, P = 16, 1024, 1024, 16, 16
D = C // H                      # 64
SCALE = float(D) ** -0.5        # 0.125
N_CORES = 8
B_PC = B // N_CORES             # 2 batches per core
NT = N // 128                   # 8 token tiles
CT = C // 128                   # 8 feature tiles
MT = NT + 1                     # 9 m-tiles: tile 0 = prefix (16 valid rows)
HPAIRS = H // 2                 # 8 head pairs


def build_nc(repeat: int = 1) -> bass.Bass:
    nc = bass.Bass()

    x_d = nc.declare_dram_parameter("x", [B_PC, N, C], F32, isOutput=False)
    pk_d = nc.declare_dram_parameter("pk", [B_PC, P, C], F32, isOutput=False)
    pv_d = nc.declare_dram_parameter("pv", [B_PC, P, C], F32, isOutput=False)
    wqkv_d = nc.declare_dram_parameter("w_qkv", [C, 3 * C], F32, isOutput=False)
    wproj_d = nc.declare_dram_parameter("w_proj", [C, C], F32, isOutput=False)
    bias_d = nc.declare_dram_parameter("b_proj", [C], F32, isOutput=False)
    # output is stored TRANSPOSED per batch: [C, N]; host transposes back
    outT_d = nc.declare_dram_parameter("outT", [B_PC, C, N], F32, isOutput=True)

    with tile.TileContext(nc) as tc:
        with tc.tile_pool(name="cons", bufs=1) as cons, \
             tc.tile_pool(name="xload", bufs=2) as xload, \
             tc.tile_pool(name="xbf", bufs=2) as xbfp, \
             tc.tile_pool(name="eP", bufs=3) as e_pool, \
             tc.tile_pool(name="stg", bufs=1) as stg, \
             tc.tile_pool(name="rbp", bufs=1) as rb_pool, \
             tc.tile_pool(name="osb", bufs=2) as osb, \
             tc.tile_pool(name="psS", bufs=2, space="PSUM") as psS, \
             tc.tile_pool(name="psAV", bufs=1, space="PSUM") as psAV, \
             tc.tile_pool(name="psG", bufs=2, space="PSUM") as psG:

            # ---------------- one-time setup ----------------
            ident_bf = cons.tile([128, 128], BF16, tag="idb")
            make_identity(nc, ident_bf[:])
            ident_f = cons.tile([128, 128], F32, tag="idf")
            make_identity(nc, ident_f[:])
            # bias in per-partition layout: bias_col[p, cf] = b_proj[cf*128+p]
            bias_col = cons.tile([128, CT], F32, tag="bias")
            nc.sync.dma_start(
                out=bias_col[:],
                in_=bias_d[:].rearrange("(a b) -> b a", b=128),
            )
            # weights, bf16, resident for the whole kernel. Loaded in
            # priority order (single sw-DGE queue serializes in emission
            # order): pair-0 q/k first, then v block 0, then the rest, so
            # batch 0's first GEMMs start ~3us in instead of ~50us.
            wq_sb = cons.tile([128, CT, C], BF16, tag="wq")
            wk_sb = cons.tile([128, CT, C], BF16, tag="wk")
            wv_sb = cons.tile([128, CT, C], BF16, tag="wv")
            wp_sb = cons.tile([128, CT, C], BF16, tag="wp")

            def _wload(dst, base, lo, hi):
                nc.gpsimd.dma_start(
                    out=dst[:, :, lo:hi],
                    in_=wqkv_d[:, base + lo:base + hi].rearrange(
                        "(ct p) f -> p ct f", p=128),
                )

            _wload(wk_sb, C, 0, 128)                  # k pair 0
            _wload(wq_sb, 0, 0, 128)                  # q pair 0
            _wload(wv_sb, 2 * C, 0, 512)              # v block 0
            _wload(wv_sb, 2 * C, 512, 1024)           # v block 1
            for p in range(1, HPAIRS):
                _wload(wk_sb, C, p * 128, (p + 1) * 128)
                _wload(wq_sb, 0, p * 128, (p + 1) * 128)
            nc.gpsimd.dma_start(
                out=wp_sb[:],
                in_=wproj_d[:].rearrange("(ct p) f -> p ct f", p=128),
            )

            # persistent activations (reused across batches; Tile tracks
            # read/write hazards on AP ranges)
            xT = cons.tile([128, CT, N], BF16, tag="xT")
            kT = cons.tile([128, CT, MT * 128], BF16, tag="kT")
            qT = cons.tile([128, CT, N], BF16, tag="qT")
            oT = cons.tile([128, CT, N], BF16, tag="oT")
            # v_ext[m, mt, h, 0:64] = v values; [.., 64:128] = ones columns
            # (denominator trick). m-tile 0 = prefix: rows 0:16 valid, rows
            # 16:128 ALL-ZERO so they contribute nothing to out or denom.
            v_ext = cons.tile([128, MT, H, 128], BF16, tag="vx")
            nc.vector.memset(v_ext[:, :, :, 64:128], 1.0)
            # prefix m-tile: zero everything (APs must start at partition
            # 0/32/64/96, so zero all 128 rows then re-set ones on rows 0:16)
            nc.vector.memset(v_ext[:, 0, :, :], 0.0)
            nc.vector.memset(v_ext[0:P, 0, :, 64:128], 1.0)
            # prefix pad columns of k: zero keys -> e = exp(0) = 1, harmless
            # because the matching v_ext rows are zero
            nc.vector.memset(kT[:, :, P:128], 0.0)

            # ---------------- per-batch work units ----------------

            def qk_units(b, p):
                """4 closures: q and k GEMMs for head pair p, split in two
                512-column halves each. Each accumulates 8 c-tiles into a
                [128,512] psum and copies (cast bf16) into qT/kT."""
                us = []
                for which in ("k", "q"):
                    for jh in range(2):
                        def u(which=which, p=p, jh=jh, b=b):
                            w_sb = wk_sb if which == "k" else wq_sb
                            ps = psG.tile([128, 512], F32, tag="g",
                                          name=f"g{which}_{b}_{p}_{jh}")
                            for ct in range(CT):
                                nc.tensor.matmul(
                                    ps[:],
                                    w_sb[:, ct, p * 128:(p + 1) * 128],
                                    xT[:, ct, jh * 512:(jh + 1) * 512],
                                    start=(ct == 0), stop=(ct == CT - 1),
                                )
                            if which == "k":
                                nc.vector.tensor_copy(
                                    kT[:, p, 128 + jh * 512:128 + (jh + 1) * 512],
                                    ps[:],
                                )
                            else:
                                nc.vector.tensor_copy(
                                    qT[:, p, jh * 512:(jh + 1) * 512], ps[:]
                                )
                        us.append(u)
                return us

            def v_units(b, bk):
                """8 closures: v GEMM for pair block bk (4 pairs = 512 v
                columns), one per token tile. x^T tile is stationary, w_v
                columns are moving -> v lands in NATURAL [token, feature]
                layout, no transpose needed."""
                us = []
                for nt in range(NT):
                    def u(nt=nt, bk=bk, b=b):
                        ps = psG.tile([128, 512], F32, tag="g",
                                      name=f"gv_{b}_{bk}_{nt}")
                        for ct in range(CT):
                            nc.tensor.matmul(
                                ps[:],
                                xT[:, ct, nt * 128:(nt + 1) * 128],
                                wv_sb[:, ct, bk * 512:(bk + 1) * 512],
                                start=(ct == 0), stop=(ct == CT - 1),
                            )
                        nc.vector.tensor_copy(
                            v_ext[:, nt + 1, 8 * bk:8 * (bk + 1), 0:64],
                            ps[:].rearrange("p (h d) -> p h d", d=64),
                        )
                    us.append(u)
                return us

            def proj_units(b):
                """8 closures: one projection f-tile pass each; emitted
                interleaved into the NEXT batch's preamble."""
                us = []
                for cf in range(CT):
                    def u(cf=cf, b=b):
                        ps = psS.tile([128, N], F32, tag="s",
                                      name=f"pp_{b}_{cf}")
                        for ct in range(CT):
                            for j in (0, 512):
                                nc.tensor.matmul(
                                    ps[:, j:j + 512],
                                    wp_sb[:, ct, cf * 128:(cf + 1) * 128],
                                    oT[:, ct, j:j + 512],
                                    start=(ct == 0), stop=(ct == CT - 1),
                                )
                        o_sb = osb.tile([128, N], F32, tag="o",
                                        name=f"osb_{b}_{cf}")
                        nc.vector.tensor_scalar_add(
                            o_sb[:], ps[:], bias_col[:, cf:cf + 1]
                        )
                        nc.sync.dma_start(
                            out=outT_d[b, cf * 128:(cf + 1) * 128, :],
                            in_=o_sb[:],
                        )
                    us.append(u)
                return us

            def emit_batch(b, carry):
                """Emit one batch; `carry` = proj closures of the previous
                batch, interleaved into this batch's preamble. Returns this
                batch's proj closures."""
                units = deque(carry)

                def drain(k=1):
                    for _ in range(k):
                        if units:
                            units.popleft()()

                # ---- preamble: x^T via bf16 PE transposes; the v GEMM for
                # token tile nt only needs xT columns nt*128..(nt+1)*128, so
                # it runs right behind transpose nt and fills the PE while
                # the next DMA/cast completes ----
                vb0 = v_units(b, 0)
                for nt in range(NT):
                    xl = xload.tile([128, C], F32, tag="xl",
                                    name=f"xl_{b}_{nt}")
                    nc.sync.dma_start(
                        out=xl[:], in_=x_d[b, nt * 128:(nt + 1) * 128, :]
                    )
                    xbf = xbfp.tile([128, C], BF16, tag="xbf",
                                    name=f"xbf_{b}_{nt}")
                    nc.scalar.activation(xbf[:], xl[:], AF.Copy)
                    ps_t = psG.tile([128, CT, 128], BF16, tag="g",
                                    name=f"pst_{b}_{nt}")
                    for ct in range(CT):
                        nc.tensor.transpose(
                            ps_t[:, ct, :],
                            xbf[:, ct * 128:(ct + 1) * 128],
                            ident_bf[:],
                        )
                    nc.vector.tensor_copy(
                        xT[:, :, nt * 128:(nt + 1) * 128], ps_t[:]
                    )
                    vb0[nt]()
                    drain(1)

                # ---- prefix: pk^T into kT cols 0:16; pv into v_ext ----
                pkl = xload.tile([128, C], F32, tag="xl", name=f"pkl_{b}")
                nc.sync.dma_start(out=pkl[0:P, :], in_=pk_d[b])
                ps_pk = psG.tile([128, CT, P], F32, tag="g",
                                 name=f"pspk_{b}")
                for ct in range(CT):
                    nc.tensor.transpose(
                        ps_pk[:, ct, :],
                        pkl[0:P, ct * 128:(ct + 1) * 128],
                        ident_f[0:P, 0:P],
                    )
                nc.vector.tensor_copy(kT[:, :, 0:P], ps_pk[:])
                nc.gpsimd.dma_start(
                    out=v_ext[0:P, 0, :, 0:64],
                    in_=pv_d[b].rearrange("p (h d) -> p h d", d=64),
                )
                drain(1)

                # ---- pair-0 q/k, interleaving the carry ----
                for u in qk_units(b, 0):
                    u()
                    drain(1)
                drain(len(units))  # force out any remaining carry

                # ---- per-head attention, gemm pipeline in the slots ----
                queue = deque()
                for p in range(HPAIRS):
                    if p + 1 < HPAIRS:
                        queue.extend(qk_units(b, p + 1))
                    if p == 0:
                        queue.extend(v_units(b, 1))
                    slot = 0
                    for hh in range(2):
                        base = hh * 64
                        h = 2 * p + hh
                        ps_av = psAV.tile([128, N], F32, tag="av",
                                          name=f"av_{b}_{h}")
                        for mt in range(MT):
                            ps_s = psS.tile([128, N], F32, tag="s",
                                            name=f"s_{b}_{h}_{mt}")
                            for j in (0, 512):
                                nc.tensor.matmul(
                                    ps_s[:, j:j + 512],
                                    kT[base:base + D, p,
                                       mt * 128:(mt + 1) * 128],
                                    qT[base:base + D, p, j:j + 512],
                                    start=True, stop=True,
                                )
                            eT = e_pool.tile([128, N], BF16, tag="e",
                                             name=f"e_{b}_{h}_{mt}")
                            nc.scalar.activation(eT[:], ps_s[:], AF.Exp,
                                                 scale=SCALE)
                            # gemm/proj filler BETWEEN exp and av: the PE
                            # would otherwise idle waiting for the exp (and,
                            # at mt==0, for the previous head's psum release)
                            slot += 1
                            if queue and (mt in (0, 5)
                                          or len(queue) >= 18 - slot):
                                queue.popleft()()
                            for j in (0, 512):
                                nc.tensor.matmul(
                                    ps_av[:, j:j + 512],
                                    v_ext[:, mt, h, :],
                                    eT[:, j:j + 512],
                                    start=(mt == 0), stop=(mt == MT - 1),
                                )
                        # normalize: out = unnorm * exp(-ln(denom)).
                        # (custom-DVE reciprocal_approx is unsupported by this
                        # walrus; iterative DVE reciprocal costs 6.5us.)
                        # The numerator is copied to SBUF so the psum
                        # accumulator is released after ~1.1us (copy || ln)
                        # instead of after the full ln->exp->mul chain.
                        num_sb = stg.tile([64, N], F32, tag="st",
                                          name=f"st_{b}_{h}")
                        nc.vector.tensor_copy(num_sb[:], ps_av[0:64, :])
                        lnd = rb_pool.tile([64, N], F32, tag="ln",
                                           name=f"ln_{b}_{h}")
                        nc.scalar.activation(lnd[:], ps_av[64:128, :], AF.Ln)
                        rb = rb_pool.tile([64, N], F32, tag="rb",
                                          name=f"rb_{b}_{h}")
                        nc.scalar.activation(rb[:], lnd[:], AF.Exp,
                                             scale=-1.0)
                        nc.vector.tensor_mul(
                            oT[base:base + D, p, :], num_sb[:], rb[:]
                        )
                    while queue:
                        queue.popleft()()

                return proj_units(b)

            carry = []
            for _rep in range(repeat):
                for b in range(B_PC):
                    carry = emit_batch(b, carry)
            for u in carry:
                u()

    return nc


_NC_CACHE = {}


def _get_nc(repeat: int = 1) -> bass.Bass:
    key = f"nc{repeat}"
    if key not in _NC_CACHE:
        _NC_CACHE[key] = build_nc(repeat)
    return _NC_CACHE[key]


def _make_runner(nc):
    """Compile the SPMD kernel ONCE into a reusable callable.

    Mirrors bass2jax.run_bass_via_pjrt's multi-core branch, but without
    output-buffer donation so the compiled function + device-resident
    inputs can be invoked repeatedly (for wall-clock benchmarking and to
    avoid recompiles on every kernel() call).
    """
    import jax
    from jax.experimental.shard_map import shard_map
    from jax.sharding import Mesh, PartitionSpec
    from concourse import bass2jax
    from concourse.bass2jax import _bass_exec_p, partition_id_tensor

    bass2jax.install_neuronx_cc_hook()

    partition_name = (
        nc.partition_id_tensor.name if nc.partition_id_tensor else None
    )
    in_names, out_names, out_avals, zero_outs = [], [], [], []
    for alloc in nc.m.functions[0].allocations:
        if not isinstance(alloc, mybir.MemoryLocationSet):
            continue
        name = alloc.memorylocations[0].name
        if alloc.kind == "ExternalInput":
            if name != partition_name:
                in_names.append(name)
        elif alloc.kind == "ExternalOutput":
            shape = tuple(alloc.tensor_shape)
            dtype = mybir.dt.np(alloc.dtype)
            out_names.append(name)
            out_avals.append(jax.core.ShapedArray(shape, dtype))
            zero_outs.append(np.zeros(shape, dtype))
    n_params = len(in_names)
    all_in_names = list(in_names) + list(out_names)
    if partition_name is not None:
        all_in_names.append(partition_name)

    def _body(*args):
        operands = list(args)
        if partition_name is not None:
            operands.append(partition_id_tensor())
        outs = _bass_exec_p.bind(
            *operands,
            out_avals=tuple(out_avals),
            in_names=tuple(all_in_names),
            out_names=tuple(out_names),
            lowering_input_output_aliases=(),
            sim_require_finite=True,
            sim_require_nnan=True,
            nc=nc,
        )
        return tuple(outs)

    devices = jax.devices()[:N_CORES]
    mesh = Mesh(np.asarray(devices), ("core",))
    n_outs = len(out_avals)
    in_specs = (PartitionSpec("core"),) * (n_params + n_outs)
    out_specs = (PartitionSpec("core"),) * n_outs
    sharded = jax.jit(
        shard_map(_body, mesh=mesh, in_specs=in_specs,
                  out_specs=out_specs, check_rep=False),
        keep_unused=True,
    )

    concat_zeros = [
        np.zeros((N_CORES * z.shape[0], *z.shape[1:]), z.dtype)
        for z in zero_outs
    ]

    state = {"dev_zeros": None}

    def runner(in_maps):
        per_core = [
            [np.asarray(m[name]) for name in in_names] for m in in_maps
        ]
        concat_in = [
            np.concatenate([per_core[c][i] for c in range(N_CORES)], axis=0)
            for i in range(n_params)
        ]
        if state["dev_zeros"] is None:
            state["dev_zeros"] = [jax.device_put(z) for z in concat_zeros]
        out_arrs = sharded(*concat_in, *state["dev_zeros"])
        return [
            {
                name: np.asarray(out_arrs[i]).reshape(
                    N_CORES, *out_avals[i].shape
                )[c]
                for i, name in enumerate(out_names)
            }
            for c in range(N_CORES)
        ]

    def runner_dev(dev_args):
        """dev_args: device-resident concat inputs; returns device outputs."""
        return sharded(*dev_args, *state["dev_zeros"])

    def make_dev_args(in_maps):
        per_core = [
            [np.asarray(m[name]) for name in in_names] for m in in_maps
        ]
        concat_in = [
            np.concatenate([per_core[c][i] for c in range(N_CORES)], axis=0)
            for i in range(n_params)
        ]
        if state["dev_zeros"] is None:
            state["dev_zeros"] = [jax.device_put(z) for z in concat_zeros]
        return [jax.device_put(a) for a in concat_in]

    return runner, runner_dev, make_dev_args


def _get_runner(repeat: int = 1):
    key = f"runner{repeat}"
    if key not in _NC_CACHE:
        _NC_CACHE[key] = _make_runner(_get_nc(repeat))
    return _NC_CACHE[key]


def _make_in_maps(x, pk, pv, w_qkv, w_proj, b_proj):
    x = np.ascontiguousarray(np.asarray(x, dtype=np.float32))
    pk = np.ascontiguousarray(np.asarray(pk, dtype=np.float32))
    pv = np.ascontiguousarray(np.asarray(pv, dtype=np.float32))
    w_qkv = np.ascontiguousarray(np.asarray(w_qkv, dtype=np.float32))
    w_proj = np.ascontiguousarray(np.asarray(w_proj, dtype=np.float32))
    b_proj = np.ascontiguousarray(np.asarray(b_proj, dtype=np.float32))
    in_maps = []
    for c in range(N_CORES):
        sl = slice(c * B_PC, (c + 1) * B_PC)
        in_maps.append({
            "x": x[sl], "pk": pk[sl], "pv": pv[sl],
            "w_qkv": w_qkv, "w_proj": w_proj, "b_proj": b_proj,
        })
    return in_maps


def run(x, pk, pv, w_qkv, w_proj, b_proj, trace=False, **trace_kwargs):
    """Run the SPMD kernel; returns (output [B,N,C], results).

    With trace=True, routes through run_bass_kernel_spmd so the returned
    results object carries .exec_time_ns / .profile_json.
    """
    in_maps = _make_in_maps(x, pk, pv, w_qkv, w_proj, b_proj)
    if trace:
        res = run_bass_kernel_spmd(
            _get_nc(), in_maps, list(range(N_CORES)), trace=True,
            **trace_kwargs,
        )
        results = res.results
        out = np.empty((B, N, C), dtype=np.float32)
        for c in range(N_CORES):
            outT = results[c]["outT"]          # [B_PC, C, N]
            out[c * B_PC:(c + 1) * B_PC] = outT.transpose(0, 2, 1)
        return out, res
    runner, _, _ = _get_runner()
    results = runner(in_maps)
    out = np.empty((B, N, C), dtype=np.float32)
    for c in range(N_CORES):
        outT = results[c]["outT"]              # [B_PC, C, N]
        out[c * B_PC:(c + 1) * B_PC] = outT.transpose(0, 2, 1)
    return out, results


def kernel(x, pk, pv, w_qkv, w_proj, b_proj) -> np.ndarray:
    out, _ = run(x, pk, pv, w_qkv, w_proj, b_proj)
    return out


def benchmark(x, pk, pv, w_qkv, w_proj, b_proj, iters=20, warmup=3, repeat=1):
    """Median wall-clock per executed call with device-resident inputs."""
    import time
    import jax
    _, runner_dev, make_dev_args = _get_runner(repeat)
    in_maps = _make_in_maps(x, pk, pv, w_qkv, w_proj, b_proj)
    dev_args = make_dev_args(in_maps)
    for _ in range(warmup):
        outs = runner_dev(dev_args)
        jax.block_until_ready(outs)
    ts = []
    for _ in range(iters):
        t0 = time.perf_counter()
        outs = runner_dev(dev_args)
        jax.block_until_ready(outs)
        ts.append(time.perf_counter() - t0)
    ts.sort()
    return {
        "median_s": ts[len(ts) // 2],
        "min_s": ts[0],
        "all_s": ts,
    }
